# revision 1
# baseline (speedup 1.0000x reference)
"""Trainium2 Bass kernel for Gumbel 2:4-masked Linear (tensor-parallel over out_features).

Math (matches the reference in forward value):
  idx    = first-occurrence argmax over 6 logits per 4-weight block,
           logits = choice_weights + gumbel_noise (choice constant -> scalar add)
  mask   = MASKING_PATTERNS[idx]          (six 2-of-4 binary patterns)
  out    = x @ (weight * mask).T + bias

Distribution: 8 NeuronCores, sharded by output rows (512 rows/core). Mask
generation and the masked GEMM are fully local; outputs concatenated on host.
x is transposed once on the host (xT [K, T]) so the GEMM's stationary operand
streams straight from DRAM with no on-chip transposes.

On-core pipeline (SPMD, per core):
  phase 1 (k-chunked): gumbel tile -> per-block max (DVE) -> exact first-max
      one-hot via prefix products of (l_p < m) compares -> mask columns by
      telescoping sums (DVE+GPSIMD, bf16 temps) -> masked weight ->
      PE-transpose into resident WmT [k, o] (float32r).
  phase 2: stream xT strips [k, 128 t] from DRAM, float32r matmuls accumulate
      psum [128 t, 512 o] over 32 k-tiles, bias add, DMA out.
"""

import numpy as np

N_CORES = 8
T = 4096          # tokens = 2*2048
K = 4096          # in_features
O_FULL = 4096     # out_features
O = O_FULL // N_CORES          # 512 out rows per core
GUM_COLS = K // 4 * 6          # 6144 logit floats per weight row
N_KC = 4                       # k chunks in phase 1
KC_K = K // N_KC               # 1024 k per chunk
KC_B = KC_K // 4               # 256 blocks per chunk row
KC_G = KC_B * 6                # 1536 logit floats per chunk row
N_KT = K // 128                # 32 k-tiles for the GEMM
N_OT = O // 128                # 4 o-tiles per core
N_TT = T // 128                # 32 token strips

_prog_cache = {}


def _build_program(mode, const_c, repeats=1):
    """mode: 'const' (choice folded to scalar) or 'full' (add choice tensor)."""
    import concourse.bacc as bacc
    import concourse.bass as bass
    import concourse.mybir as mybir
    import concourse.tile as tile
    from concourse.masks import make_identity

    f32 = mybir.dt.float32
    f32r = mybir.dt.float32r
    bf16 = mybir.dt.bfloat16
    Alu = mybir.AluOpType

    nc = bacc.Bacc(trn_type="TRN2")
    xt_d = nc.declare_dram_parameter("xt", [K, T], f32r, isOutput=False)
    w_d = nc.declare_dram_parameter("w", [O, K], f32, isOutput=False)
    b_d = nc.declare_dram_parameter("b", [1, O], f32, isOutput=False)
    g_d = nc.declare_dram_parameter("g", [O, GUM_COLS], f32, isOutput=False)
    if mode == "full":
        cw_d = nc.declare_dram_parameter("cw", [O, GUM_COLS], f32, isOutput=False)
    out_d = nc.declare_dram_parameter("out", [T, O], f32, isOutput=True)
    # [K, T] viewed as [kp=128, kt=32, t] for per-strip loads
    xt_v = xt_d.rearrange("(a p) t -> p a t", p=128)

    with tile.TileContext(nc) as tc:
        with (
            tc.tile_pool(name="singles", bufs=1) as singles,
            tc.tile_pool(name="wmt", bufs=N_KC) as wmt_pool,
            tc.tile_pool(name="gum", bufs=2) as gum_pool,
            tc.tile_pool(name="wtile", bufs=2) as w_pool,
            tc.tile_pool(name="mtmp", bufs=2) as mtmp,
            tc.tile_pool(name="xt", bufs=4) as xt_pool,
            tc.tile_pool(name="outs", bufs=2) as out_pool,
            tc.tile_pool(name="ps_xpose", bufs=1, space="PSUM") as ps_xpose,
            tc.tile_pool(name="ps_gemm", bufs=6, space="PSUM") as ps_gemm,
        ):
            ident_f32 = singles.tile([128, 128], f32)
            make_identity(nc, ident_f32)
            ident = singles.tile([128, 128], f32r, name="ident_r")
            nc.scalar.copy(ident, ident_f32)
            bias_s = singles.tile([128, O], f32)
            nc.gpsimd.dma_start(
                out=bias_s,
                in_=bass.AP(tensor=b_d, offset=0, ap=[[0, 128], [1, O]]),
            )

            # resident transposed masked weight, one tile per k chunk:
            # wmt[kc][p=k%128, j=kt within chunk, o]
            wmt = [
                wmt_pool.tile([128, N_KC * 2, O], f32r, name=f"wmt{i}", tag=f"wmt{i}", bufs=1)
                for i in range(N_KC)
            ]

            for _rep in range(repeats):
                # ------------- phase 1: mask + masked weight + transpose ----
                for kc in range(N_KC):
                    for ot in range(N_OT):
                        rows = slice(ot * 128, (ot + 1) * 128)
                        g_t = gum_pool.tile([128, KC_G], f32, tag="gum")
                        nc.sync.dma_start(
                            out=g_t, in_=g_d[rows, kc * KC_G:(kc + 1) * KC_G]
                        )
                        if mode == "full":
                            cw_t = gum_pool.tile([128, KC_G], f32, tag="cw")
                            nc.sync.dma_start(
                                out=cw_t, in_=cw_d[rows, kc * KC_G:(kc + 1) * KC_G]
                            )
                            nc.vector.tensor_add(g_t, g_t, cw_t)
                        elif const_c != 0.0:
                            nc.vector.tensor_scalar_add(g_t, g_t, const_c)
                        g3 = g_t.rearrange("p (b s) -> p b s", s=6)

                        m = mtmp.tile([128, KC_B], f32, tag="m")
                        nc.vector.tensor_reduce(
                            m, g3, axis=mybir.AxisListType.X, op=Alu.max
                        )

                        # exact first-max one-hot via prefix products
                        # c1=f0, c_{p+1}=c_p*f_p with f_p=(l_p<m); e0=(l0>=m)
                        e0 = mtmp.tile([128, KC_B], bf16, tag="e0")
                        c1 = mtmp.tile([128, KC_B], bf16, tag="c1")
                        nc.vector.tensor_tensor(c1, g3[:, :, 0], m, op=Alu.is_lt)
                        nc.vector.tensor_tensor(e0, g3[:, :, 0], m, op=Alu.is_ge)
                        f1 = mtmp.tile([128, KC_B], bf16, tag="f1")
                        f2 = mtmp.tile([128, KC_B], bf16, tag="f2")
                        f3 = mtmp.tile([128, KC_B], bf16, tag="f3")
                        f4 = mtmp.tile([128, KC_B], bf16, tag="f4")
                        nc.vector.tensor_tensor(f1, g3[:, :, 1], m, op=Alu.is_lt)
                        nc.vector.tensor_tensor(f2, g3[:, :, 2], m, op=Alu.is_lt)
                        nc.vector.tensor_tensor(f3, g3[:, :, 3], m, op=Alu.is_lt)
                        nc.vector.tensor_tensor(f4, g3[:, :, 4], m, op=Alu.is_lt)
                        c2 = mtmp.tile([128, KC_B], bf16, tag="c2")
                        c3 = mtmp.tile([128, KC_B], bf16, tag="c3")
                        c4 = mtmp.tile([128, KC_B], bf16, tag="c4")
                        c5 = mtmp.tile([128, KC_B], bf16, tag="c5")
                        nc.gpsimd.tensor_mul(c2, c1, f1)
                        nc.gpsimd.tensor_mul(c3, c2, f2)
                        nc.gpsimd.tensor_mul(c4, c3, f3)
                        nc.gpsimd.tensor_mul(c5, c4, f4)

                        w_t = w_pool.tile([128, KC_K], f32, tag="w")
                        nc.sync.dma_start(
                            out=w_t, in_=w_d[rows, kc * KC_K:(kc + 1) * KC_K]
                        )
                        w4 = w_t.rearrange("p (b s) -> p b s", s=4)
                        wm = w_pool.tile([128, KC_K], f32r, tag="wm")
                        wm4 = wm.rearrange("p (b s) -> p b s", s=4)

                        t0 = mtmp.tile([128, KC_B], bf16, tag="t0")
                        t1 = mtmp.tile([128, KC_B], bf16, tag="t1")
                        t2 = mtmp.tile([128, KC_B], bf16, tag="t2")
                        # col0 = c3
                        nc.vector.tensor_mul(wm4[:, :, 0], w4[:, :, 0], c3)
                        # col1 = c1 - c3 + c5
                        nc.gpsimd.tensor_sub(t0, c1, c3)
                        nc.gpsimd.tensor_add(t0, t0, c5)
                        nc.vector.tensor_mul(wm4[:, :, 1], w4[:, :, 1], t0)
                        # col2 = e0 + (c2-c3) + (c4-c5)
                        nc.gpsimd.tensor_sub(t1, c2, c3)
                        nc.gpsimd.tensor_sub(t2, c4, c5)
                        nc.gpsimd.tensor_add(t1, t1, t2)
                        nc.gpsimd.tensor_add(t1, t1, e0)
                        nc.vector.tensor_mul(wm4[:, :, 2], w4[:, :, 2], t1)
                        # col3 = e0 + (c1-c2) + (c3-c4)
                        nc.vector.tensor_sub(t2, c1, c2)
                        nc.vector.tensor_sub(t0, c3, c4)
                        nc.vector.tensor_add(t2, t2, t0)
                        nc.vector.tensor_add(t2, t2, e0)
                        nc.vector.tensor_mul(wm4[:, :, 3], w4[:, :, 3], t2)

                        # transpose 8 [128,128] subtiles -> wmt[kc][:, :, ot*128:]
                        ps = ps_xpose.tile([128, 1024], f32r, tag="psx")
                        for j in range(8):
                            nc.tensor.transpose(
                                ps[:, j * 128:(j + 1) * 128],
                                wm[:, j * 128:(j + 1) * 128],
                                ident,
                            )
                        nc.scalar.copy(
                            wmt[kc][:, :, ot * 128:(ot + 1) * 128],
                            ps.rearrange("p (a b) -> p a b", a=8),
                        )

                # ------------- phase 2: stream xT, GEMM --------------------
                for tt in range(N_TT):
                    trows = slice(tt * 128, (tt + 1) * 128)
                    xt = xt_pool.tile([128, N_KT, 128], f32r, tag="xt")
                    nc.sync.dma_start(out=xt, in_=xt_v[:, :, trows])

                    acc = ps_gemm.tile([128, O], f32, tag="acc")
                    for kc in range(N_KC):
                        for j in range(N_KC * 2):
                            kt = kc * (N_KC * 2) + j
                            nc.tensor.matmul(
                                acc,
                                xt[:, kt, :],
                                wmt[kc][:, j, :],
                                start=(kt == 0),
                                stop=(kt == N_KT - 1),
                            )
                    o_t = out_pool.tile([128, O], f32, tag="o")
                    nc.vector.tensor_add(o_t, acc, bias_s)
                    nc.sync.dma_start(out=out_d[trows, :], in_=o_t)

    nc.compile()
    return nc


def _get_program(mode, const_c):
    key = (mode, const_c)
    if key not in _prog_cache:
        _prog_cache[key] = _build_program(mode, const_c)
    return _prog_cache[key]


def kernel(x, weight, bias, choice_weights, gumbel_noise):
    from concourse.bass_utils import run_bass_kernel_spmd

    x = np.asarray(x, dtype=np.float32).reshape(T, K)
    xt = np.ascontiguousarray(x.T)
    w = np.ascontiguousarray(np.asarray(weight, dtype=np.float32))
    b = np.ascontiguousarray(np.asarray(bias, dtype=np.float32)).reshape(1, O_FULL)
    cw = np.asarray(choice_weights, dtype=np.float32)
    g = np.asarray(gumbel_noise, dtype=np.float32).reshape(O_FULL, GUM_COLS)

    c0 = float(cw.flat[0])
    is_const = bool((cw == c0).all())
    mode = "const" if is_const else "full"
    nc = _get_program(mode, c0 if is_const else None)

    in_maps = []
    for c in range(N_CORES):
        rows = slice(c * O, (c + 1) * O)
        m = {
            "xt": xt,
            "w": np.ascontiguousarray(w[rows]),
            "b": np.ascontiguousarray(b[:, rows]),
            "g": np.ascontiguousarray(g[rows]),
        }
        if mode == "full":
            m["cw"] = np.ascontiguousarray(
                cw.reshape(O_FULL, GUM_COLS)[rows]
            )
        in_maps.append(m)

    res = run_bass_kernel_spmd(nc, in_maps, list(range(N_CORES)))
    parts = [res.results[c]["out"] for c in range(N_CORES)]
    out = np.concatenate(parts, axis=1)  # [T, O_FULL]
    return out.reshape(2, 2048, O_FULL)



# revision 5
# speedup vs baseline: 1.2822x; 1.2822x over previous
"""Trainium2 Bass kernel for Gumbel 2:4-masked Linear (tensor-parallel over out_features).

Math (matches the reference in forward value):
  idx    = argmax over 6 logits per 4-weight block
           (logits = choice_weights + gumbel_noise; constant choice -> shift
            does not change the argmax, so it is skipped in 'const' mode)
  mask   = MASKING_PATTERNS[idx]          (six 2-of-4 binary patterns)
  out    = x @ (weight * mask).T + bias

Distribution: 8 NeuronCores, sharded by output rows (512 rows/core). Mask
generation and the masked GEMM are fully local; outputs concatenated on host.

Mask math: pattern j is the edge {p,q} of K4 with 1s at positions p,q; the set
S_p of patterns with a 1 at position p is {3,4,5}/{1,2,5}/{0,2,4}/{0,1,3}.
mask[p] = [max_{j in S_p} g_j >= max_j g_j] — 10 pairwise maxes + 4 compares
per block-tile, no softmax / one-hot needed (exact except on fp32 ties,
which are measure-zero).

Schedule (per core):
  - GEMM operands in bf16 (tolerance 2e-2; halves x/w HBM traffic), psum f32.
  - x host-packed as [128 part, tt, kt, 128] so every token-strip load is one
    contiguous 8KB/partition DMA.
  - phase 1 is chunked by (kc, ot); units alternate between DVE and GpSimd.
  - W warmup token strips hold psum banks and accumulate each k-chunk's
    matmuls as soon as its transposed masked weight lands, hiding mask
    generation of later chunks behind PE work; remaining strips stream.
"""

import numpy as np

N_CORES = 8
T = 4096          # tokens = 2*2048
K = 4096          # in_features
O_FULL = 4096     # out_features
O = O_FULL // N_CORES          # 512 out rows per core
GUM_COLS = K // 4 * 6          # 6144 logit floats per weight row
N_KC = 4                       # k chunks in phase 1
KC_K = K // N_KC               # 1024 k per chunk
KC_B = KC_K // 4               # 256 blocks per chunk row
KC_G = KC_B * 6                # 1536 logit floats per chunk row
N_KT = K // 128                # 32 k-tiles for the GEMM
N_OT = O // 128                # 4 o-tiles per core
N_TT = T // 128                # 32 token strips
N_JT = N_KT // N_KC            # 8 k-tiles per chunk
W = 6                          # warmup strips overlapped with phase 1

_prog_cache = {}


def _build_program(mode):
    """mode: 'const' (constant choice_weights folded away) or 'full'."""
    import concourse.bacc as bacc
    import concourse.bass as bass
    import concourse.mybir as mybir
    import concourse.tile as tile
    from concourse.masks import make_identity

    f32 = mybir.dt.float32
    bf16 = mybir.dt.bfloat16
    Alu = mybir.AluOpType

    nc = bacc.Bacc(trn_type="TRN2")
    xt_d = nc.declare_dram_parameter("xt", [128, N_TT * K], bf16, isOutput=False)
    w_d = nc.declare_dram_parameter("w", [O, K], bf16, isOutput=False)
    b_d = nc.declare_dram_parameter("b", [1, O], f32, isOutput=False)
    g_d = nc.declare_dram_parameter("g", [O, GUM_COLS], f32, isOutput=False)
    if mode == "full":
        cw_d = nc.declare_dram_parameter("cw", [O, GUM_COLS], f32, isOutput=False)
    out_d = nc.declare_dram_parameter("out", [T, O], f32, isOutput=True)
    xt_v = xt_d.rearrange("p (t c) -> p t c", t=N_TT)

    with tile.TileContext(nc) as tc:
        with (
            tc.tile_pool(name="singles", bufs=1) as singles,
            tc.tile_pool(name="wmt", bufs=N_KC) as wmt_pool,
            tc.tile_pool(name="gum", bufs=3) as gum_pool,
            tc.tile_pool(name="wtile", bufs=2) as w_pool,
            tc.tile_pool(name="mtmp", bufs=2) as mtmp,
            tc.tile_pool(name="xt", bufs=9) as xt_pool,
            tc.tile_pool(name="outs", bufs=4) as out_pool,
            tc.tile_pool(name="ps_x", bufs=2, space="PSUM") as ps_x,
            tc.tile_pool(name="ps_acc", bufs=6, space="PSUM") as ps_acc,
        ):
            ident_f32 = singles.tile([128, 128], f32)
            make_identity(nc, ident_f32)
            ident = singles.tile([128, 128], bf16, name="ident_bf")
            nc.scalar.copy(ident, ident_f32)
            bias_s = singles.tile([128, O], f32)
            nc.gpsimd.dma_start(
                out=bias_s,
                in_=bass.AP(tensor=b_d, offset=0, ap=[[0, 128], [1, O]]),
            )

            # resident transposed masked weight: wmt[kc][p=k%128, j, o]
            wmt = [
                wmt_pool.tile([128, N_JT, O], bf16, name=f"wmt{i}", tag=f"wmt{i}", bufs=1)
                for i in range(N_KC)
            ]

            def mask_unit(kc, ot, eng):
                """Masked weight for rows [128*ot, 128*(ot+1)), cols kc-chunk,
                transposed into wmt[kc]. Elementwise work on engine `eng`."""
                E = nc.vector if eng == "v" else nc.gpsimd
                rows = slice(ot * 128, (ot + 1) * 128)
                g_t = gum_pool.tile([128, KC_G], f32, tag="gum")
                nc.sync.dma_start(
                    out=g_t, in_=g_d[rows, kc * KC_G:(kc + 1) * KC_G]
                )
                if mode == "full":
                    cw_t = gum_pool.tile([128, KC_G], f32, tag="cw")
                    nc.sync.dma_start(
                        out=cw_t, in_=cw_d[rows, kc * KC_G:(kc + 1) * KC_G]
                    )
                    E.tensor_add(g_t, g_t, cw_t)
                g3 = g_t.rearrange("p (b s) -> p b s", s=6)

                # group maxes M_p = max over S_p, global max mx
                tmp = [
                    mtmp.tile([128, KC_B], f32, tag=f"t{i}{eng}", name=f"t{i}{eng}", bufs=1)
                    for i in range(6)
                ]
                a0, a3, M0, M1, M2, M3 = tmp
                E.tensor_tensor(a0, g3[:, :, 3], g3[:, :, 4], op=Alu.max)
                E.tensor_tensor(M0, a0, g3[:, :, 5], op=Alu.max)
                E.tensor_tensor(M1, g3[:, :, 1], g3[:, :, 2], op=Alu.max)
                E.tensor_tensor(M1, M1, g3[:, :, 5], op=Alu.max)
                E.tensor_tensor(M2, g3[:, :, 0], g3[:, :, 2], op=Alu.max)
                E.tensor_tensor(M2, M2, g3[:, :, 4], op=Alu.max)
                E.tensor_tensor(a3, g3[:, :, 0], g3[:, :, 1], op=Alu.max)
                E.tensor_tensor(M3, a3, g3[:, :, 3], op=Alu.max)
                mx = mtmp.tile([128, KC_B], f32, tag=f"mx{eng}", bufs=1)
                E.tensor_tensor(mx, a3, g3[:, :, 2], op=Alu.max)
                E.tensor_tensor(mx, mx, M0, op=Alu.max)

                mask = mtmp.tile([128, KC_K], bf16, tag=f"mask{eng}")
                m4 = mask.rearrange("p (b s) -> p b s", s=4)
                E.tensor_tensor(m4[:, :, 0], M0, mx, op=Alu.is_ge)
                E.tensor_tensor(m4[:, :, 1], M1, mx, op=Alu.is_ge)
                E.tensor_tensor(m4[:, :, 2], M2, mx, op=Alu.is_ge)
                E.tensor_tensor(m4[:, :, 3], M3, mx, op=Alu.is_ge)

                w_t = w_pool.tile([128, KC_K], bf16, tag="w")
                nc.sync.dma_start(
                    out=w_t, in_=w_d[rows, kc * KC_K:(kc + 1) * KC_K]
                )
                wm = w_pool.tile([128, KC_K], bf16, tag="wm")
                nc.vector.tensor_mul(wm, w_t, mask)

                ps = ps_x.tile([128, KC_K], bf16, tag="psx")
                for j in range(N_JT):
                    nc.tensor.transpose(
                        ps[:, j * 128:(j + 1) * 128],
                        wm[:, j * 128:(j + 1) * 128],
                        ident,
                    )
                nc.scalar.copy(
                    wmt[kc][:, :, ot * 128:(ot + 1) * 128],
                    ps.rearrange("p (a b) -> p a b", a=N_JT),
                )

            x_tiles = {}

            def load_x(tt):
                xs = xt_pool.tile([128, K], bf16, tag="x")
                nc.sync.dma_start(out=xs, in_=xt_v[:, tt, :])
                x_tiles[tt] = xs.rearrange("p (a b) -> p a b", b=128)

            def burst(tt, kc, acc):
                xs3 = x_tiles[tt]
                for j in range(N_JT):
                    kt = kc * N_JT + j
                    nc.tensor.matmul(
                        acc,
                        xs3[:, kt, :],
                        wmt[kc][:, j, :],
                        start=(kt == 0),
                        stop=(kt == N_KT - 1),
                    )

            def drain(tt, acc, eng):
                E = nc.vector if eng == "v" else nc.gpsimd
                o_t = out_pool.tile([128, O], f32, tag="o")
                E.tensor_add(o_t, acc, bias_s)
                nc.sync.dma_start(
                    out=out_d[tt * 128:(tt + 1) * 128, :], in_=o_t
                )

            engs = ["v", "g", "v", "g"]

            # ---- phase A: masks per k-chunk, warmup strips accumulate ----
            warm_acc = {}
            for kc in range(N_KC):
                for ot in range(N_OT):
                    mask_unit(kc, ot, engs[(kc + ot) % 4])
                if kc == 0:
                    for s in range(W):
                        load_x(s)
                for s in range(W):
                    if kc == 0:
                        warm_acc[s] = ps_acc.tile([128, O], f32, tag="acc", name=f"acc{s}")
                    burst(s, kc, warm_acc[s])
            for s in range(W):
                drain(s, warm_acc[s], engs[s % 2])

            # ---- phase B: stream remaining strips over resident wmt ----
            for tt in range(W, N_TT):
                load_x(tt)
                acc = ps_acc.tile([128, O], f32, tag="acc")
                for kc in range(N_KC):
                    burst(tt, kc, acc)
                drain(tt, acc, engs[tt % 2])

    nc.compile()
    return nc


def _get_program(mode, const_c=None):
    if mode not in _prog_cache:
        _prog_cache[mode] = _build_program(mode)
    return _prog_cache[mode]


def pack_inputs(x, weight, bias, choice_weights, gumbel_noise):
    """Host-side prep: returns (mode, per-core input maps)."""
    from concourse import mybir

    bf16 = mybir.dt.np(mybir.dt.bfloat16)

    x = np.asarray(x, dtype=np.float32).reshape(T, K)
    # [tt, i, a, p] -> [p, tt, a, i]: each strip is contiguous per partition
    xp = x.reshape(N_TT, 128, N_KT, 128).transpose(3, 0, 2, 1)
    xt = np.ascontiguousarray(xp).astype(bf16).reshape(128, N_TT * K)
    w = np.asarray(weight, dtype=np.float32).astype(bf16)
    b = np.ascontiguousarray(np.asarray(bias, dtype=np.float32)).reshape(1, O_FULL)
    cw = np.asarray(choice_weights, dtype=np.float32)
    g = np.asarray(gumbel_noise, dtype=np.float32).reshape(O_FULL, GUM_COLS)

    is_const = bool((cw == cw.flat[0]).all())
    mode = "const" if is_const else "full"

    in_maps = []
    for c in range(N_CORES):
        rows = slice(c * O, (c + 1) * O)
        m = {
            "xt": xt,
            "w": np.ascontiguousarray(w[rows]),
            "b": np.ascontiguousarray(b[:, rows]),
            "g": np.ascontiguousarray(g[rows]),
        }
        if mode == "full":
            m["cw"] = np.ascontiguousarray(cw.reshape(O_FULL, GUM_COLS)[rows])
        in_maps.append(m)
    return mode, in_maps


def kernel(x, weight, bias, choice_weights, gumbel_noise):
    from concourse.bass_utils import run_bass_kernel_spmd

    mode, in_maps = pack_inputs(x, weight, bias, choice_weights, gumbel_noise)
    nc = _get_program(mode)
    res = run_bass_kernel_spmd(nc, in_maps, list(range(N_CORES)))
    parts = [res.results[c]["out"] for c in range(N_CORES)]
    out = np.concatenate(parts, axis=1)  # [T, O_FULL]
    return out.reshape(2, 2048, O_FULL)


# revision 21
# speedup vs baseline: 1.3300x; 1.0373x over previous
"""Trainium2 Bass kernel for Gumbel 2:4-masked Linear (tensor-parallel over out_features).

Math (matches the reference in forward value):
  idx    = argmax over 6 logits per 4-weight block
           (logits = choice_weights + gumbel_noise; constant choice -> shift
            does not change the argmax, so it is skipped in 'const' mode)
  mask   = MASKING_PATTERNS[idx]          (six 2-of-4 binary patterns)
  out    = x @ (weight * mask).T + bias

Distribution: 8 NeuronCores, sharded by output rows (512 rows/core). Mask
generation and the masked GEMM are fully local; outputs concatenated on host.

Mask math: pattern j is the edge {p,q} of K4 with 1s at positions p,q; the set
S_p of patterns with a 1 at position p is {3,4,5}/{1,2,5}/{0,2,4}/{0,1,3}.
mask[p] = [max_{j in S_p} g_j >= max_j g_j] — 10 pairwise maxes + 4 compares
per block-tile, no softmax / one-hot needed (exact except on fp32 ties,
which are measure-zero).

Schedule (per core):
  - GEMM operands in bf16 (tolerance 2e-2; halves x/w HBM traffic), psum f32.
  - x host-packed as [128 part, tt, kt, 128] so every token-strip load is one
    contiguous 8KB/partition DMA.
  - phase 1 is chunked by (kc, ot); units alternate between DVE and GpSimd.
  - W warmup token strips hold psum banks and accumulate each k-chunk's
    matmuls as soon as its transposed masked weight lands, hiding mask
    generation of later chunks behind PE work; remaining strips stream.
"""

import numpy as np

N_CORES = 8
T = 4096          # tokens = 2*2048
K = 4096          # in_features
O_FULL = 4096     # out_features
O = O_FULL // N_CORES          # 512 out rows per core
GUM_COLS = K // 4 * 6          # 6144 logit floats per weight row
N_KC = 4                       # k chunks in phase 1
KC_K = K // N_KC               # 1024 k per chunk
KC_B = KC_K // 4               # 256 blocks per chunk row
KC_G = KC_B * 6                # 1536 logit floats per chunk row
N_KT = K // 128                # 32 k-tiles for the GEMM
N_OT = O // 128                # 4 o-tiles per core
N_TT = T // 128                # 32 token strips
N_JT = N_KT // N_KC            # 8 k-tiles per chunk
V = 9                          # windowed strips overlapped with phase 1

_prog_cache = {}


def _build_program(mode):
    """mode: 'const' (constant choice_weights folded away) or 'full'."""
    import concourse.bacc as bacc
    import concourse.bass as bass
    import concourse.mybir as mybir
    import concourse.tile as tile
    from concourse.masks import make_identity

    f32 = mybir.dt.float32
    bf16 = mybir.dt.bfloat16
    Alu = mybir.AluOpType

    nc = bacc.Bacc(trn_type="TRN2")
    xt_d = nc.declare_dram_parameter("xt", [128, N_TT * K], bf16, isOutput=False)
    w_d = nc.declare_dram_parameter("w", [O, K], bf16, isOutput=False)
    b_d = nc.declare_dram_parameter("b", [1, O], f32, isOutput=False)
    g_d = nc.declare_dram_parameter("g", [O, GUM_COLS], f32, isOutput=False)
    if mode == "full":
        cw_d = nc.declare_dram_parameter("cw", [O, GUM_COLS], f32, isOutput=False)
    out_d = nc.declare_dram_parameter("out", [T, O], f32, isOutput=True)
    xt_v = xt_d.rearrange("p (t c) -> p t c", t=N_TT)

    with tile.TileContext(nc) as tc:
        with (
            tc.tile_pool(name="singles", bufs=1) as singles,
            tc.tile_pool(name="wmt", bufs=N_KC) as wmt_pool,
            tc.tile_pool(name="gum", bufs=8) as gum_pool,
            tc.tile_pool(name="wtile", bufs=4) as w_pool,
            tc.tile_pool(name="mtmp", bufs=2) as mtmp,
            tc.tile_pool(name="xt", bufs=9) as xt_pool,
            tc.tile_pool(name="osb", bufs=1) as osb_pool,
            tc.tile_pool(name="outs", bufs=4) as out_pool,
            tc.tile_pool(name="ps_x", bufs=2, space="PSUM") as ps_x,
            tc.tile_pool(name="ps_warm", bufs=3, space="PSUM") as ps_warm,
            tc.tile_pool(name="ps_acc", bufs=3, space="PSUM") as ps_acc,
        ):
            ident_f32 = singles.tile([128, 128], f32)
            make_identity(nc, ident_f32)
            ident = singles.tile([128, 128], bf16, name="ident_bf")
            nc.scalar.copy(ident, ident_f32)
            bias_s = singles.tile([128, O], f32)
            nc.gpsimd.dma_start(
                out=bias_s,
                in_=bass.AP(tensor=b_d, offset=0, ap=[[0, 128], [1, O]]),
            )

            # resident transposed masked weight: wmt[kc][p=k%128, j, o]
            wmt = [
                wmt_pool.tile([128, N_JT, O], bf16, name=f"wmt{i}", tag=f"wmt{i}", bufs=1)
                for i in range(N_KC)
            ]

            g_tiles = {}
            w_tiles = {}

            def g_dma(kc, ot, q):
                rows = slice(ot * 128, (ot + 1) * 128)
                g_t = gum_pool.tile([128, KC_G], f32, tag="gum", name=f"g{kc}{ot}")
                q.dma_start(out=g_t, in_=g_d[rows, kc * KC_G:(kc + 1) * KC_G])
                g_tiles[(kc, ot)] = g_t
                if mode == "full":
                    cw_t = gum_pool.tile([128, KC_G], f32, tag="cw", name=f"c{kc}{ot}")
                    q.dma_start(out=cw_t, in_=cw_d[rows, kc * KC_G:(kc + 1) * KC_G])
                    g_tiles[(kc, ot, "cw")] = cw_t

            def w_dma(kc, ot, q):
                rows = slice(ot * 128, (ot + 1) * 128)
                w_t = w_pool.tile([128, KC_K], bf16, tag="w", name=f"w{kc}{ot}")
                q.dma_start(out=w_t, in_=w_d[rows, kc * KC_K:(kc + 1) * KC_K])
                w_tiles[(kc, ot)] = w_t

            def mask_unit(kc, ot, eng):
                """Masked weight for rows [128*ot, 128*(ot+1)), cols kc-chunk,
                transposed into wmt[kc].

                Host delivers gumbel PLANAR per chunk (6 planes of KC_B) with
                plane order [g4,g3,g2,g1,g0,g5], so fused pair ops (DVE) use
                ascending contiguous-inner slices and plain per-plane ops
                (GpSimd) are fully contiguous. Planes: 0=g4 1=g3 2=g2 3=g1
                4=g0 5=g5.  S_p sets: col0={3,4,5} col1={1,2,5} col2={0,2,4}
                col3={0,1,3} (original indices)."""
                g_t = g_tiles[(kc, ot)]
                if mode == "full":
                    E0 = nc.vector if eng == "v" else nc.gpsimd
                    E0.tensor_add(g_t, g_t, g_tiles[(kc, ot, "cw")])
                gs = g_t.rearrange("p (s b) -> p s b", s=6)
                w_t = w_tiles[(kc, ot)]
                wm = w_pool.tile([128, KC_K], bf16, tag="wm", bufs=2)

                # max/compare ops exist only on DVE (Pool ALU: add/sub/mult);
                # fused paired ops: u2=[M0|M1], u4=[M2|M3]
                u2 = mtmp.tile([128, 2, KC_B], f32, tag="u2v", bufs=2)
                u4 = mtmp.tile([128, 2, KC_B], f32, tag="u4v", bufs=2)
                V = nc.vector
                V.tensor_tensor(u2, gs[:, 1:4:2, :], gs[:, 0:3:2, :], op=Alu.max)
                g5b = gs[:, 5:6, :].broadcast_to([128, 2, KC_B])
                V.tensor_tensor(u2, u2, g5b, op=Alu.max)      # [M0|M1]
                g0b = gs[:, 4:5, :].broadcast_to([128, 2, KC_B])
                V.tensor_tensor(u4, g0b, gs[:, 2:4, :], op=Alu.max)
                V.tensor_tensor(u4, u4, gs[:, 0:2, :], op=Alu.max)  # [M2|M3]
                mx = mtmp.tile([128, KC_B], f32, tag="mxv", bufs=2)
                V.tensor_tensor(mx, u2[:, 0, :], gs[:, 2, :], op=Alu.max)
                V.tensor_tensor(mx, mx, u4[:, 1, :], op=Alu.max)
                # interleaved mask -> packed bf16 mul (on Pool: plain mult)
                mask = mtmp.tile([128, KC_K], bf16, tag="maskv", bufs=2)
                mv = mask.rearrange("p (b s) -> p s b", s=4)
                mxb = mx.unsqueeze(1).broadcast_to([128, 2, KC_B])
                V.tensor_tensor(mv[:, 0:2, :], u2, mxb, op=Alu.is_ge)
                V.tensor_tensor(mv[:, 2:4, :], u4, mxb, op=Alu.is_ge)
                MU = nc.gpsimd if eng == "g" else nc.vector
                MU.tensor_mul(wm, w_t, mask)

                ps = ps_x.tile([128, KC_K], bf16, tag="psx")
                for j in range(N_JT):
                    nc.tensor.transpose(
                        ps[:, j * 128:(j + 1) * 128],
                        wm[:, j * 128:(j + 1) * 128],
                        ident,
                    )
                nc.scalar.copy(
                    wmt[kc][:, :, ot * 128:(ot + 1) * 128],
                    ps.rearrange("p (a b) -> p a b", a=N_JT),
                )

            x_tiles = {}

            def load_x(tt):
                xs = xt_pool.tile([128, K], bf16, tag="x")
                nc.sync.dma_start(out=xs, in_=xt_v[:, tt, :])
                x_tiles[tt] = xs.rearrange("p (a b) -> p a b", b=128)

            def pass_burst(tt, kc):
                """One k-chunk's 8 mms for strip tt into a fresh psum tile
                (windowed strips: accumulated into SBUF between passes)."""
                xs3 = x_tiles[tt]
                acc = ps_acc.tile([128, O], f32, tag="acc", name=f"pw{tt}_{kc}")
                for j in range(N_JT):
                    nc.tensor.matmul(
                        acc,
                        xs3[:, kc * N_JT + j, :],
                        wmt[kc][:, j, :],
                        start=(j == 0),
                        stop=(j == N_JT - 1),
                    )
                return acc

            def full_burst(tt):
                """All 32 mms for strip tt into one psum accumulation."""
                xs3 = x_tiles[tt]
                acc = ps_acc.tile([128, O], f32, tag="acc", name=f"pf{tt}")
                for kc in range(N_KC):
                    for j in range(N_JT):
                        kt = kc * N_JT + j
                        nc.tensor.matmul(
                            acc,
                            xs3[:, kt, :],
                            wmt[kc][:, j, :],
                            start=(kt == 0),
                            stop=(kt == N_KT - 1),
                        )
                return acc

            def out_dma(tt, o_t):
                nc.scalar.dma_start(
                    out=out_d[tt * 128:(tt + 1) * 128, :], in_=o_t
                )

            def drain(tt, acc, eng, split=False):
                # psum reads are DVE-only (GPSIMD cannot access PSUM)
                o_t = out_pool.tile([128, O], f32, tag="o", name=f"o{tt}")
                if split:
                    nc.vector.tensor_add(o_t[:, :O // 2], acc[:, :O // 2],
                                         bias_s[:, :O // 2])
                    nc.vector.tensor_add(o_t[:, O // 2:], acc[:, O // 2:],
                                         bias_s[:, O // 2:])
                else:
                    nc.vector.tensor_add(o_t, acc, bias_s)
                out_dma(tt, o_t)

            engs = ["v", "g"]
            queues = [nc.sync, nc.scalar]
            WARM = 3                 # psum-resident strips (kc bursts x4)
            # window strips WARM..V-1: two 2-chunk passes (kc01 -> osb,
            # kc23 -> +bias +osb -> out), so no per-chunk adds serialize the
            # engines against the next chunk's mask units.

            # ---- DMA prefetch stream in need-order -----------------------
            for kc in (0, 1):
                for ot in range(N_OT):
                    g_dma(kc, ot, queues[ot % 2])
            for kc in (0, 1):
                for ot in range(N_OT):
                    w_dma(kc, ot, queues[ot % 2])
            for sidx in range(3):
                load_x(sidx)
            for ot in range(N_OT):
                g_dma(2, ot, queues[ot % 2])
            for ot in range(N_OT):
                w_dma(2, ot, queues[ot % 2])
            for sidx in range(3, 6):
                load_x(sidx)
            for ot in range(N_OT):
                g_dma(3, ot, queues[ot % 2])
            for ot in range(N_OT):
                w_dma(3, ot, queues[ot % 2])
            for sidx in range(6, V):
                load_x(sidx)

            osb = {}
            warm_acc = {}

            def warm_burst(tt, kc):
                xs3 = x_tiles[tt]
                if kc == 0:
                    warm_acc[tt] = ps_warm.tile([128, O], f32, tag="wacc",
                                                name=f"wa{tt}")
                acc = warm_acc[tt]
                for j in range(N_JT):
                    kt = kc * N_JT + j
                    nc.tensor.matmul(
                        acc,
                        xs3[:, kt, :],
                        wmt[kc][:, j, :],
                        start=(kt == 0),
                        stop=(kt == N_KT - 1),
                    )

            def window_pass(i, half):
                """half 0: chunks 0-1 -> osb; half 1: chunks 2-3 -> out."""
                xs3 = x_tiles[i]
                acc = ps_acc.tile([128, O], f32, tag="acc", name=f"pw{i}_{half}")
                for n in range(2 * N_JT):
                    kc = half * 2 + n // N_JT
                    j = n % N_JT
                    nc.tensor.matmul(
                        acc,
                        xs3[:, kc * N_JT + j, :],
                        wmt[kc][:, j, :],
                        start=(n == 0),
                        stop=(n == 2 * N_JT - 1),
                    )
                if half == 0:
                    osb[i] = osb_pool.tile([128, O], bf16, name=f"osb{i}",
                                           tag=f"osb{i}")
                    nc.scalar.copy(osb[i], acc)
                else:
                    o_t = out_pool.tile([128, O], f32, tag="o", name=f"ow{i}")
                    nc.vector.tensor_add(o_t, acc, bias_s)
                    nc.gpsimd.tensor_add(o_t, o_t, osb[i])
                    out_dma(i, o_t)

            # ---- phase A ------------------------------------------------
            for ot in range(N_OT):
                mask_unit(0, ot, engs[ot % 2])
            for ot in range(N_OT):
                mask_unit(1, ot, engs[(1 + ot) % 2])
            for s in range(WARM):
                warm_burst(s, 0)
            for ot in range(N_OT):
                mask_unit(2, ot, engs[ot % 2])
            for s in range(WARM):
                warm_burst(s, 1)
            for i in range(WARM, V):
                window_pass(i, 0)
            for ot in range(N_OT):
                mask_unit(3, ot, engs[(1 + ot) % 2])
            for s in range(WARM):
                warm_burst(s, 2)
            for s in range(WARM):
                warm_burst(s, 3)
            for i in range(WARM, V):
                window_pass(i, 1)
            for s in range(WARM):
                drain(s, warm_acc[s], engs[s % 2])

            # ---- phase B: stream remaining strips over resident wmt ------
            for tt in range(V, N_TT):
                load_x(tt)
                acc = full_burst(tt)
                drain(tt, acc, engs[tt % 2], split=(tt == N_TT - 1))

    nc.compile()
    return nc


def _get_program(mode, const_c=None):
    if mode not in _prog_cache:
        _prog_cache[mode] = _build_program(mode)
    return _prog_cache[mode]


def pack_inputs(x, weight, bias, choice_weights, gumbel_noise):
    """Host-side prep: returns (mode, per-core input maps)."""
    from concourse import mybir

    bf16 = mybir.dt.np(mybir.dt.bfloat16)

    x = np.asarray(x, dtype=np.float32).reshape(T, K)
    # [tt, i, a, p] -> [p, tt, a, i]: each strip is contiguous per partition
    xp = x.reshape(N_TT, 128, N_KT, 128).transpose(3, 0, 2, 1)
    xt = np.ascontiguousarray(xp).astype(bf16).reshape(128, N_TT * K)
    w = np.asarray(weight, dtype=np.float32).astype(bf16)
    b = np.ascontiguousarray(np.asarray(bias, dtype=np.float32)).reshape(1, O_FULL)
    # device mask math expects planes [g4,g3,g2,g1,g0,g5], planar per chunk:
    # per (row, chunk): [KC_B blocks, 6] -> [6 planes, KC_B]
    PERM = [4, 3, 2, 1, 0, 5]

    def planarize(a):
        a = np.asarray(a, dtype=np.float32)[:, PERM]
        a = a.reshape(O_FULL, N_KC, KC_B, 6).transpose(0, 1, 3, 2)
        return np.ascontiguousarray(a).reshape(O_FULL, GUM_COLS)

    cw = np.asarray(choice_weights, dtype=np.float32)
    g = planarize(gumbel_noise)

    is_const = bool((cw == cw.flat[0]).all())
    mode = "const" if is_const else "full"

    in_maps = []
    for c in range(N_CORES):
        rows = slice(c * O, (c + 1) * O)
        m = {
            "xt": xt,
            "w": np.ascontiguousarray(w[rows]),
            "b": np.ascontiguousarray(b[:, rows]),
            "g": np.ascontiguousarray(g[rows]),
        }
        if mode == "full":
            m["cw"] = np.ascontiguousarray(planarize(cw)[rows])
        in_maps.append(m)
    return mode, in_maps


def kernel(x, weight, bias, choice_weights, gumbel_noise):
    from concourse.bass_utils import run_bass_kernel_spmd

    mode, in_maps = pack_inputs(x, weight, bias, choice_weights, gumbel_noise)
    nc = _get_program(mode)
    res = run_bass_kernel_spmd(nc, in_maps, list(range(N_CORES)))
    parts = [res.results[c]["out"] for c in range(N_CORES)]
    out = np.concatenate(parts, axis=1)  # [T, O_FULL]
    return out.reshape(2, 2048, O_FULL)


# revision 30
# speedup vs baseline: 1.3392x; 1.0069x over previous
"""Trainium2 Bass kernel for Gumbel 2:4-masked Linear (tensor-parallel over out_features).

Math (matches the reference in forward value):
  idx    = argmax over 6 logits per 4-weight block
           (logits = choice_weights + gumbel_noise; constant choice -> shift
            does not change the argmax, so it is skipped in 'const' mode)
  mask   = MASKING_PATTERNS[idx]          (six 2-of-4 binary patterns)
  out    = x @ (weight * mask).T + bias

Distribution: 8 NeuronCores, sharded by output rows (512 rows/core). Mask
generation and the masked GEMM are fully local; outputs concatenated on host.

Mask math: pattern j is the edge {p,q} of K4 with 1s at positions p,q; the set
S_p of patterns with a 1 at position p is {3,4,5}/{1,2,5}/{0,2,4}/{0,1,3}.
mask[p] = [max_{j in S_p} g_j >= max_j g_j] — 10 pairwise maxes + 4 compares
per block-tile, no softmax / one-hot needed (exact except on fp32 ties,
which are measure-zero).

Schedule (per core):
  - GEMM operands in bf16 (tolerance 2e-2; halves x/w HBM traffic), psum f32.
  - x host-packed as [128 part, tt, kt, 128] so every token-strip load is one
    contiguous 8KB/partition DMA.
  - phase 1 is chunked by (kc, ot); units alternate between DVE and GpSimd.
  - W warmup token strips hold psum banks and accumulate each k-chunk's
    matmuls as soon as its transposed masked weight lands, hiding mask
    generation of later chunks behind PE work; remaining strips stream.
"""

import numpy as np

N_CORES = 8
T = 4096          # tokens = 2*2048
K = 4096          # in_features
O_FULL = 4096     # out_features
O = O_FULL // N_CORES          # 512 out rows per core
GUM_COLS = K // 4 * 6          # 6144 logit floats per weight row
N_KC = 4                       # k chunks in phase 1
KC_K = K // N_KC               # 1024 k per chunk
KC_B = KC_K // 4               # 256 blocks per chunk row
KC_G = KC_B * 6                # 1536 logit floats per chunk row
N_KT = K // 128                # 32 k-tiles for the GEMM
N_OT = O // 128                # 4 o-tiles per core
N_TT = T // 128                # 32 token strips
N_JT = N_KT // N_KC            # 8 k-tiles per chunk
V = 9                          # windowed strips overlapped with phase 1

_prog_cache = {}


def _build_program(mode):
    """mode: 'const' (constant choice_weights folded away) or 'full'."""
    import concourse.bacc as bacc
    import concourse.bass as bass
    import concourse.mybir as mybir
    import concourse.tile as tile
    from concourse.masks import make_identity

    f32 = mybir.dt.float32
    bf16 = mybir.dt.bfloat16
    Alu = mybir.AluOpType

    nc = bacc.Bacc(trn_type="TRN2")
    xt_d = nc.declare_dram_parameter("xt", [128, N_TT * K], bf16, isOutput=False)
    w_d = nc.declare_dram_parameter("w", [O, K], bf16, isOutput=False)
    b_d = nc.declare_dram_parameter("b", [1, O], f32, isOutput=False)
    g_d = nc.declare_dram_parameter("g", [O, GUM_COLS], f32, isOutput=False)
    if mode == "full":
        cw_d = nc.declare_dram_parameter("cw", [O, GUM_COLS], f32, isOutput=False)
    out_d = nc.declare_dram_parameter("out", [T, O], f32, isOutput=True)
    xt_v = xt_d.rearrange("p (t c) -> p t c", t=N_TT)

    with tile.TileContext(nc) as tc:
        with (
            tc.tile_pool(name="singles", bufs=1) as singles,
            tc.tile_pool(name="wmt", bufs=N_KC) as wmt_pool,
            tc.tile_pool(name="gum", bufs=8) as gum_pool,
            tc.tile_pool(name="wtile", bufs=4) as w_pool,
            tc.tile_pool(name="mtmp", bufs=2) as mtmp,
            tc.tile_pool(name="xt", bufs=9) as xt_pool,
            tc.tile_pool(name="osb", bufs=1) as osb_pool,
            tc.tile_pool(name="outs", bufs=4) as out_pool,
            tc.tile_pool(name="ps_x", bufs=2, space="PSUM") as ps_x,
            tc.tile_pool(name="ps_warm", bufs=3, space="PSUM") as ps_warm,
            tc.tile_pool(name="ps_acc", bufs=3, space="PSUM") as ps_acc,
        ):
            ident_f32 = singles.tile([128, 128], f32)
            make_identity(nc, ident_f32)
            ident = singles.tile([128, 128], bf16, name="ident_bf")
            nc.scalar.copy(ident, ident_f32)
            bias_s = singles.tile([128, O], f32)
            nc.gpsimd.dma_start(
                out=bias_s,
                in_=bass.AP(tensor=b_d, offset=0, ap=[[0, 128], [1, O]]),
            )

            # resident transposed masked weight: wmt[kc][p=k%128, j, o]
            wmt = [
                wmt_pool.tile([128, N_JT, O], bf16, name=f"wmt{i}", tag=f"wmt{i}", bufs=1)
                for i in range(N_KC)
            ]

            g_tiles = {}
            w_tiles = {}

            def g_dma(kc, ot, q):
                rows = slice(ot * 128, (ot + 1) * 128)
                g_t = gum_pool.tile([128, KC_G], f32, tag="gum", name=f"g{kc}{ot}")
                q.dma_start(out=g_t, in_=g_d[rows, kc * KC_G:(kc + 1) * KC_G])
                g_tiles[(kc, ot)] = g_t
                if mode == "full":
                    cw_t = gum_pool.tile([128, KC_G], f32, tag="cw", name=f"c{kc}{ot}")
                    q.dma_start(out=cw_t, in_=cw_d[rows, kc * KC_G:(kc + 1) * KC_G])
                    g_tiles[(kc, ot, "cw")] = cw_t

            def w_dma(kc, ot, q):
                rows = slice(ot * 128, (ot + 1) * 128)
                w_t = w_pool.tile([128, KC_K], bf16, tag="w", name=f"w{kc}{ot}")
                q.dma_start(out=w_t, in_=w_d[rows, kc * KC_K:(kc + 1) * KC_K])
                w_tiles[(kc, ot)] = w_t

            def mask_unit(kc, ot, eng):
                """Masked weight for rows [128*ot, 128*(ot+1)), cols kc-chunk,
                transposed into wmt[kc].

                Host delivers gumbel PLANAR per chunk (6 planes of KC_B) with
                plane order [g4,g3,g2,g1,g0,g5], so fused pair ops (DVE) use
                ascending contiguous-inner slices and plain per-plane ops
                (GpSimd) are fully contiguous. Planes: 0=g4 1=g3 2=g2 3=g1
                4=g0 5=g5.  S_p sets: col0={3,4,5} col1={1,2,5} col2={0,2,4}
                col3={0,1,3} (original indices)."""
                g_t = g_tiles[(kc, ot)]
                if mode == "full":
                    E0 = nc.vector if eng == "v" else nc.gpsimd
                    E0.tensor_add(g_t, g_t, g_tiles[(kc, ot, "cw")])
                gs = g_t.rearrange("p (s b) -> p s b", s=6)
                w_t = w_tiles[(kc, ot)]
                wm = w_pool.tile([128, KC_K], bf16, tag="wm", bufs=2)

                # max/compare ops exist only on DVE (Pool ALU: add/sub/mult);
                # fused paired ops: u2=[M0|M1], u4=[M2|M3]
                u2 = mtmp.tile([128, 2, KC_B], f32, tag="u2v", bufs=2)
                u4 = mtmp.tile([128, 2, KC_B], f32, tag="u4v", bufs=2)
                V = nc.vector
                V.tensor_tensor(u2, gs[:, 1:4:2, :], gs[:, 0:3:2, :], op=Alu.max)
                g5b = gs[:, 5:6, :].broadcast_to([128, 2, KC_B])
                V.tensor_tensor(u2, u2, g5b, op=Alu.max)      # [M0|M1]
                g0b = gs[:, 4:5, :].broadcast_to([128, 2, KC_B])
                V.tensor_tensor(u4, g0b, gs[:, 2:4, :], op=Alu.max)
                V.tensor_tensor(u4, u4, gs[:, 0:2, :], op=Alu.max)  # [M2|M3]
                mx = mtmp.tile([128, KC_B], f32, tag="mxv", bufs=2)
                V.tensor_tensor(mx, u2[:, 0, :], gs[:, 2, :], op=Alu.max)
                V.tensor_tensor(mx, mx, u4[:, 1, :], op=Alu.max)
                # compares off DVE: e_p = M_p - mx  (Pool), then
                # col_p = Relu(e_p * 1e14 + 1)      (ACT) — e_p <= 0, and
                # e_p == 0 iff the argmax pattern covers position p.
                e2 = mtmp.tile([128, 2, KC_B], f32, tag="e2", bufs=2)
                e4 = mtmp.tile([128, 2, KC_B], f32, tag="e4", bufs=2)
                P = nc.gpsimd
                P.tensor_sub(e2[:, 0, :], u2[:, 0, :], mx)
                P.tensor_sub(e2[:, 1, :], u2[:, 1, :], mx)
                P.tensor_sub(e4[:, 0, :], u4[:, 0, :], mx)
                P.tensor_sub(e4[:, 1, :], u4[:, 1, :], mx)
                mask = mtmp.tile([128, KC_K], bf16, tag="maskv", bufs=2)
                mv = mask.rearrange("p (b s) -> p s b", s=4)
                Relu = mybir.ActivationFunctionType.Relu
                nc.scalar.activation(mv[:, 0:2, :], e2, Relu, bias=1.0, scale=1e14)
                nc.scalar.activation(mv[:, 2:4, :], e4, Relu, bias=1.0, scale=1e14)
                MU = nc.gpsimd if eng == "g" else nc.vector
                MU.tensor_mul(wm, w_t, mask)

                ps = ps_x.tile([128, KC_K], bf16, tag="psx")
                for j in range(N_JT):
                    nc.tensor.transpose(
                        ps[:, j * 128:(j + 1) * 128],
                        wm[:, j * 128:(j + 1) * 128],
                        ident,
                    )
                nc.scalar.copy(
                    wmt[kc][:, :, ot * 128:(ot + 1) * 128],
                    ps.rearrange("p (a b) -> p a b", a=N_JT),
                )

            x_tiles = {}

            def load_x(tt):
                xs = xt_pool.tile([128, K], bf16, tag="x")
                nc.sync.dma_start(out=xs, in_=xt_v[:, tt, :])
                x_tiles[tt] = xs.rearrange("p (a b) -> p a b", b=128)

            def pass_burst(tt, kc):
                """One k-chunk's 8 mms for strip tt into a fresh psum tile
                (windowed strips: accumulated into SBUF between passes)."""
                xs3 = x_tiles[tt]
                acc = ps_acc.tile([128, O], f32, tag="acc", name=f"pw{tt}_{kc}")
                for j in range(N_JT):
                    nc.tensor.matmul(
                        acc,
                        xs3[:, kc * N_JT + j, :],
                        wmt[kc][:, j, :],
                        start=(j == 0),
                        stop=(j == N_JT - 1),
                    )
                return acc

            def full_burst(tt, halves=False):
                """All 32 mms for strip tt into one psum accumulation.
                halves=True: two o-half chains so the first half's drain and
                store overlap the second half's matmuls (tail strips)."""
                xs3 = x_tiles[tt]
                acc = ps_acc.tile([128, O], f32, tag="acc", name=f"pf{tt}")
                for osl in ([slice(0, O // 2), slice(O // 2, O)]
                            if halves else [slice(None)]):
                    for kc in range(N_KC):
                        for j in range(N_JT):
                            kt = kc * N_JT + j
                            nc.tensor.matmul(
                                acc[:, osl],
                                xs3[:, kt, :],
                                wmt[kc][:, j, osl],
                                start=(kt == 0),
                                stop=(kt == N_KT - 1),
                            )
                return acc

            def out_dma(tt, o_t):
                nc.sync.dma_start(
                    out=out_d[tt * 128:(tt + 1) * 128, :], in_=o_t
                )

            def drain(tt, acc, eng, split=False):
                # psum reads are DVE-only (GPSIMD cannot access PSUM)
                o_t = out_pool.tile([128, O], f32, tag="o", name=f"o{tt}")
                if split:
                    nc.vector.tensor_add(o_t[:, :O // 2], acc[:, :O // 2],
                                         bias_s[:, :O // 2])
                    nc.vector.tensor_add(o_t[:, O // 2:], acc[:, O // 2:],
                                         bias_s[:, O // 2:])
                else:
                    nc.vector.tensor_add(o_t, acc, bias_s)
                out_dma(tt, o_t)

            engs = ["v", "g"]
            queues = [nc.sync, nc.sync]
            WARM = 3                 # psum-resident strips (kc bursts x4)
            # window strips WARM..V-1: two 2-chunk passes (kc01 -> osb,
            # kc23 -> +bias +osb -> out), so no per-chunk adds serialize the
            # engines against the next chunk's mask units.

            # ---- DMA prefetch stream in need-order -----------------------
            # w(kc) rides right behind g(kc) (the mask mul needs it); x strips
            # interleave just-in-time for warm/window bursts.
            def gw(kc):
                for ot in range(N_OT):
                    g_dma(kc, ot, queues[ot % 2])
                for ot in range(N_OT):
                    w_dma(kc, ot, queues[ot % 2])

            gw(0)
            gw(1)
            for sidx in range(0, 3):
                load_x(sidx)
            gw(2)
            for sidx in range(3, 5):
                load_x(sidx)
            gw(3)
            for sidx in range(5, V):
                load_x(sidx)

            osb = {}
            warm_acc = {}

            def warm_burst(tt, kc):
                xs3 = x_tiles[tt]
                if kc == 0:
                    warm_acc[tt] = ps_warm.tile([128, O], f32, tag="wacc",
                                                name=f"wa{tt}")
                acc = warm_acc[tt]
                for j in range(N_JT):
                    kt = kc * N_JT + j
                    nc.tensor.matmul(
                        acc,
                        xs3[:, kt, :],
                        wmt[kc][:, j, :],
                        start=(kt == 0),
                        stop=(kt == N_KT - 1),
                    )

            def window_pass(i, half):
                """half 0: chunks 0-1 -> osb; half 1: chunks 2-3 -> out."""
                xs3 = x_tiles[i]
                acc = ps_acc.tile([128, O], f32, tag="acc", name=f"pw{i}_{half}")
                for n in range(2 * N_JT):
                    kc = half * 2 + n // N_JT
                    j = n % N_JT
                    nc.tensor.matmul(
                        acc,
                        xs3[:, kc * N_JT + j, :],
                        wmt[kc][:, j, :],
                        start=(n == 0),
                        stop=(n == 2 * N_JT - 1),
                    )
                if half == 0:
                    osb[i] = osb_pool.tile([128, O], bf16, name=f"osb{i}",
                                           tag=f"osb{i}")
                    nc.scalar.copy(osb[i], acc)
                else:
                    o_t = out_pool.tile([128, O], f32, tag="o", name=f"ow{i}")
                    nc.vector.tensor_add(o_t, acc, bias_s)
                    nc.gpsimd.tensor_add(o_t, o_t, osb[i])
                    out_dma(i, o_t)

            # ---- phase A ------------------------------------------------
            for ot in range(N_OT):
                mask_unit(0, ot, engs[ot % 2])
            for ot in range(N_OT):
                mask_unit(1, ot, engs[(1 + ot) % 2])
            for s in range(WARM):
                warm_burst(s, 0)
            for ot in range(N_OT):
                mask_unit(2, ot, engs[ot % 2])
            for s in range(WARM):
                warm_burst(s, 1)
            for i in range(WARM, V):
                window_pass(i, 0)
            for ot in range(N_OT):
                mask_unit(3, ot, engs[(1 + ot) % 2])
            for s in range(WARM):
                warm_burst(s, 2)
            for s in range(WARM):
                warm_burst(s, 3)
            for i in range(WARM, V):
                window_pass(i, 1)
            for s in range(WARM):
                drain(s, warm_acc[s], engs[s % 2])

            # ---- phase B: stream remaining strips over resident wmt ------
            for tt in range(V, N_TT):
                load_x(tt)
                if tt == N_TT - 1:
                    acc = full_burst(tt, halves=True)
                    o_t = out_pool.tile([128, O], f32, tag="o", name="olast")
                    H = O // 2
                    nc.vector.tensor_add(o_t[:, :H], acc[:, :H], bias_s[:, :H])
                    nc.sync.dma_start(out=out_d[tt * 128:(tt + 1) * 128, :H],
                                       in_=o_t[:, :H])
                    nc.vector.tensor_add(o_t[:, H:], acc[:, H:], bias_s[:, H:])
                    nc.sync.dma_start(out=out_d[tt * 128:(tt + 1) * 128, H:],
                                      in_=o_t[:, H:])
                else:
                    acc = full_burst(tt)
                    drain(tt, acc, engs[tt % 2])

    nc.compile()
    return nc


def _get_program(mode, const_c=None):
    if mode not in _prog_cache:
        _prog_cache[mode] = _build_program(mode)
    return _prog_cache[mode]


def pack_inputs(x, weight, bias, choice_weights, gumbel_noise):
    """Host-side prep: returns (mode, per-core input maps)."""
    from concourse import mybir

    bf16 = mybir.dt.np(mybir.dt.bfloat16)

    x = np.asarray(x, dtype=np.float32).reshape(T, K)
    # [tt, i, a, p] -> [p, tt, a, i]: each strip is contiguous per partition
    xp = x.reshape(N_TT, 128, N_KT, 128).transpose(3, 0, 2, 1)
    xt = np.ascontiguousarray(xp).astype(bf16).reshape(128, N_TT * K)
    w = np.asarray(weight, dtype=np.float32).astype(bf16)
    b = np.ascontiguousarray(np.asarray(bias, dtype=np.float32)).reshape(1, O_FULL)
    # device mask math expects planes [g4,g3,g2,g1,g0,g5], planar per chunk:
    # per (row, chunk): [KC_B blocks, 6] -> [6 planes, KC_B]
    PERM = [4, 3, 2, 1, 0, 5]

    def planarize(a):
        a = np.asarray(a, dtype=np.float32)[:, PERM]
        a = a.reshape(O_FULL, N_KC, KC_B, 6).transpose(0, 1, 3, 2)
        return np.ascontiguousarray(a).reshape(O_FULL, GUM_COLS)

    cw = np.asarray(choice_weights, dtype=np.float32)
    g = planarize(gumbel_noise)

    is_const = bool((cw == cw.flat[0]).all())
    mode = "const" if is_const else "full"

    in_maps = []
    for c in range(N_CORES):
        rows = slice(c * O, (c + 1) * O)
        m = {
            "xt": xt,
            "w": np.ascontiguousarray(w[rows]),
            "b": np.ascontiguousarray(b[:, rows]),
            "g": np.ascontiguousarray(g[rows]),
        }
        if mode == "full":
            m["cw"] = np.ascontiguousarray(planarize(cw)[rows])
        in_maps.append(m)
    return mode, in_maps


def kernel(x, weight, bias, choice_weights, gumbel_noise):
    from concourse.bass_utils import run_bass_kernel_spmd

    mode, in_maps = pack_inputs(x, weight, bias, choice_weights, gumbel_noise)
    nc = _get_program(mode)
    res = run_bass_kernel_spmd(nc, in_maps, list(range(N_CORES)))
    parts = [res.results[c]["out"] for c in range(N_CORES)]
    out = np.concatenate(parts, axis=1)  # [T, O_FULL]
    return out.reshape(2, 2048, O_FULL)
